# revision 22
# baseline (speedup 1.0000x reference)
"""Multi-head self-attention (B=4, N=2048, D=1024, H=16) on 8 trn2 NeuronCores.

Sharding: 8 shards = (batch, head-half).  Core c handles batch c//2 and heads
[(c%2)*8, (c%2)*8+8) -- tensor parallel over heads: w_q/w_k/w_v column-sliced
by head, w_o row-sliced; the partial-output all-reduce of the tensor-parallel
unshard is folded into the host-side gather together with the bias add.

Host-side input marshalling (layout only, no FLOPs): tensors are pre-packed
into DMA-native layouts (partition-major, slice-contiguous) and cast to bf16
so every priority slice lands at full HBM bandwidth:
  z  -> [128, 8 col-blocks, 8 d-chunks, 256]   (256-column blocks)
  wq/wk -> [128, 4 oc, 8 dc, 128]              (per-128-dout slices)
  wv -> [128, 8 dc, 512], wo -> [128, 4 hc, 1024] (single straight loads)

Per-core kernel (Tile), all SBUF-resident, one flat (q-half, head, key-chunk)
pipeline with a 4-step scores->PV skew.  Scores use a 64-row contraction
(K^T/Q^T live in 64-partition tiles; no zero padding).  The input DMAs are
issued on four queues (sync/gpsimd/vector/scalar) in consumption order and
the first K/Q projection tiles are emitted in 256-column sub-tiles so the PE
starts ~6us earlier than a whole-tile gate would allow.

Two psum phases: while the projection fills run (through qh0 head 5) the
scores ring is 2x[128,1024] + a 2x[128,512] projection pool; from qh0 head 6
the projection pool is released and the ring becomes 2x[128,1536] with one
exp per 1536 columns -- the wider activation drops ACT time per iteration
below the PE's scores+PV rate, so the back half of the kernel stays PE-bound
instead of ACT-bound.  Fill schedule: V' chunks ride qh0 head 0 (consumed at
skew 4), K/Q projection pairs ride qh0 heads 1-5 against their deadlines,
the Q sh1 (q-half-1) projections ride qh0 h6-h7 / qh1 h1 / qh1 h3 on the
scores ring, and the 8 qh0 out-projection chunks spread across qh1; the 8
qh1 out-projection chunks drain in the tail over three DMA queues with the
last chunk's copy+store split in half to shorten the serial tail.
"""

import os
import sys

_TRN_REPO = "/opt/trn_rl_repo"
if os.path.isdir(_TRN_REPO) and _TRN_REPO not in sys.path:
    sys.path.insert(0, _TRN_REPO)

import ml_dtypes
import numpy as np

import concourse.bass as bass  # noqa: E402
import concourse.mybir as mybir  # noqa: E402
from concourse import bacc  # noqa: E402
from concourse.bass_utils import run_bass_kernel_spmd  # noqa: E402
from concourse.tile import TileContext  # noqa: E402

F32 = mybir.dt.float32
BF16 = mybir.dt.bfloat16
MULT = mybir.AluOpType.mult
EXP = mybir.ActivationFunctionType.Exp

N_CORES = 8
B, N, D = 4, 2048, 1024
H, HD = 16, 64
HL = 8            # heads per core
DH = HL * HD      # 512 local attn dims
P = 128
DC = D // P       # 8 din chunks
OC = DH // P      # 4 local dout chunks (2 heads each)
HC = OC
NKC = N // P      # 16 key chunks
NB = 16           # 128-col z blocks
NQH = N // 2      # 1024 queries per half
SCALE = 1.0 / 8.0  # 1/sqrt(HD)
SKEW = 4          # scores -> PV pipeline depth (iterations)
PH2_F = 96        # first flat iteration of the 1536-wide-exp phase
BF = ml_dtypes.bfloat16


def _build():
    nc = bacc.Bacc("TRN2", target_bir_lowering=False, debug=False,
                   num_devices=N_CORES)
    zt_d = nc.declare_dram_parameter("zt", [P, NB, DC, P], BF16,
                                     isOutput=False)
    wq_d = nc.declare_dram_parameter("wq", [P, OC, DC, P], BF16,
                                     isOutput=False)
    wk_d = nc.declare_dram_parameter("wk", [P, OC, DC, P], BF16,
                                     isOutput=False)
    wv_d = nc.declare_dram_parameter("wv", [P, DC, DH], BF16, isOutput=False)
    wo_d = nc.declare_dram_parameter("wo", [P, HC, D], BF16, isOutput=False)
    out_d = nc.declare_dram_parameter("out", [N, D], BF16, isOutput=True)

    with TileContext(nc) as tc:
        pp = tc.alloc_tile_pool(name="persist", bufs=1)
        # Per-head scores operands: head h in partitions 0-63 of slot h,
        # partitions 64-127 zero (full 128-row contraction)
        ktp = pp.tile([P, HL, N], BF16)
        qtp = pp.tile([P, HL, N], BF16)
        # V' = [V_h | 1] per head: [keys 128, key-chunk, head, 65] bf16
        vp = pp.tile([P, NKC, HL, HD + 1], BF16)
        attnT = pp.tile([P, HC, N], BF16)
        wo_sb = pp.tile([P, HC, D], BF16)

        pvp = tc.alloc_tile_pool(name="pvo", bufs=1, space="PSUM")
        ssp = tc.alloc_tile_pool(name="pss", bufs=2, space="PSUM")
        esp = tc.alloc_tile_pool(name="es", bufs=6)
        nrm = tc.alloc_tile_pool(name="nrm", bufs=2)
        outp = tc.alloc_tile_pool(name="ot", bufs=3)

        zp = tc.alloc_tile_pool(name="zin", bufs=1)
        wp = tc.alloc_tile_pool(name="wts", bufs=1)
        # SBUF layouts mirror the DRAM packs so every DMA slice is >=2KB
        # contiguous on BOTH sides (fragmented destinations run at 1/4 BW)
        zt_sb = zp.tile([P, NB, DC, P], BF16)
        wv_sb = wp.tile([P, DC, DH], BF16)
        wk_sb = wp.tile([P, OC, DC, P], BF16)
        wq_sb = wp.tile([P, OC, DC, P], BF16)

        def z_block(eng, cb):
            eng.dma_start(zt_sb[:, cb], zt_d[:, cb])

        def w_oc(eng, w_sb, w_d, oc):
            eng.dma_start(w_sb[:, oc], w_d[:, oc])

        def z_rhs(dc, c0, w):
            # [128, w] moving operand over z columns [c0, c0+w), c0%128==0
            return zt_sb[:, c0 // P:(c0 + w) // P, dc, :]

        def z_lhs(dc, kc):
            # [128, 128] stationary operand over z columns [kc*128, +128)
            return zt_sb[:, kc, dc, :]

        # input DMAs in consumption order across the three DMA-capable
        # queues (sync/gpsimd/scalar): the first K projection tile needs
        # only wk-oc0 + z block 0 (0.75 MB), so the PE starts as soon as
        # those land
        # strict global priority over two queues; the scalar queue stays
        # empty so low-priority pulls never steal bandwidth from the
        # preloop's 2.5MB working set
        w_oc(nc.sync, wk_sb, wk_d, 0)
        z_block(nc.gpsimd, 0)
        nc.vector.memset(vp[:, :, :, HD], 1.0)
        nc.vector.memset(qtp[64:P, 0, :], 0.0)
        nc.vector.memset(ktp[64:P, 0, :], 0.0)
        for bb in (1, 3):
            z_block(nc.sync, bb)
        z_block(nc.gpsimd, 2)
        z_block(nc.gpsimd, 4)
        w_oc(nc.gpsimd, wq_sb, wq_d, 0)
        z_block(nc.sync, 5)
        z_block(nc.sync, 7)
        z_block(nc.gpsimd, 6)
        z_block(nc.gpsimd, 8)
        nc.sync.dma_start(wv_sb[:], wv_d[:])
        z_block(nc.gpsimd, 10)
        z_block(nc.sync, 9)
        z_block(nc.sync, 11)
        z_block(nc.gpsimd, 12)
        z_block(nc.gpsimd, 14)
        z_block(nc.sync, 13)
        z_block(nc.sync, 15)
        w_oc(nc.gpsimd, wk_sb, wk_d, 1)
        w_oc(nc.sync, wq_sb, wq_d, 1)
        w_oc(nc.gpsimd, wk_sb, wk_d, 2)
        w_oc(nc.sync, wq_sb, wq_d, 2)
        w_oc(nc.gpsimd, wk_sb, wk_d, 3)
        w_oc(nc.sync, wq_sb, wq_d, 3)
        nc.gpsimd.dma_start(wo_sb[:], wo_d[:])

        p1p = tc.alloc_tile_pool(name="psp1", bufs=2, space="PSUM")
        pools = {"scores": ssp}

        def kq_half(w_sb, dst, oc, sh, q2, sub=1, ring=False):
            # one [128 dout, 512 seq] projection tile -> dst head slots
            # 2*oc / 2*oc+1.  sub=2 emits two 256-col accumulation groups so
            # the earliest tiles start on a single landed z block.
            s0 = sh * 1024 + q2 * 512
            if ring:
                big = pools["scores"].tile([P, 1536], F32, name="pss",
                                           tag="pss")
                ps = big[:, 0:512]
            else:
                ps = p1p.tile([P, 512], F32, name="p1")
            w = 512 // sub
            for s in range(sub):
                for dc in range(DC):
                    nc.tensor.matmul(
                        ps[:, s * w:(s + 1) * w],
                        lhsT=w_sb[:, oc, dc, :],
                        rhs=z_rhs(dc, s0 + s * w, w),
                        start=(dc == 0), stop=(dc == DC - 1))
            nc.vector.tensor_copy(dst[0:64, 2 * oc, s0:s0 + 512],
                                  ps[0:64, :])
            nc.vector.tensor_copy(dst[0:64, 2 * oc + 1, s0:s0 + 512],
                                  ps[64:P, :])

        def v_chunk(kc):
            ps = p1p.tile([P, DH], F32, name="p1")
            for dc in range(DC):
                nc.tensor.matmul(
                    ps[:],
                    lhsT=z_lhs(dc, kc),
                    rhs=wv_sb[:, dc, :],
                    start=(dc == 0), stop=(dc == DC - 1))
            nc.vector.tensor_copy(
                vp[:, kc, :, 0:HD], ps.rearrange("p (h d) -> p h d", d=HD))

        def outproj_chunk(q8, split_finish=False):
            big = pools["scores"].tile([P, 1536], F32, name="pof", tag="pss")
            psf = big[:, 0:D]
            for oc2 in range(2):
                for dc in range(HC):
                    nc.tensor.matmul(
                        psf[:, oc2 * 512:(oc2 + 1) * 512],
                        lhsT=attnT[:, dc, q8 * P:(q8 + 1) * P],
                        rhs=wo_sb[:, dc, oc2 * 512:(oc2 + 1) * 512],
                        start=(dc == 0), stop=(dc == HC - 1))
            ot = outp.tile([P, D], BF16)
            if split_finish:
                for hh in range(2):
                    nc.vector.tensor_copy(ot[:, hh * 512:(hh + 1) * 512],
                                          psf[:, hh * 512:(hh + 1) * 512])
                    eng = (nc.sync, nc.scalar)[hh]
                    eng.dma_start(
                        out_d[q8 * P:(q8 + 1) * P, hh * 512:(hh + 1) * 512],
                        ot[:, hh * 512:(hh + 1) * 512])
                return
            nc.vector.tensor_copy(ot[:], psf[:])
            if q8 < 8:
                eng = nc.sync
            else:
                eng = (nc.sync, nc.gpsimd, nc.scalar)[q8 % 3]
            eng.dma_start(out_d[q8 * P:(q8 + 1) * P, :], ot[:])

        def normalize(qh, h, pso, split=1):
            # free the PV psum fast (two copies), normalize off-path.
            # denominator goes to a partition-0 tile: reciprocal_approx_fast
            # miscomputes on partition-offset inputs.
            q0 = qh * NQH
            po = nrm.tile([HD, NQH], F32, tag="po", bufs=1)
            den = nrm.tile([1, NQH], F32, tag="den", bufs=1)
            rec = nrm.tile([1, NQH], F32, tag="rec", bufs=1)
            rb = nrm.tile([64, NQH], F32, tag="rb", bufs=1)
            w = NQH // split
            pr = 64 * (h % 2)
            for s in range(split):
                sl = slice(s * w, (s + 1) * w)
                nc.vector.tensor_copy(den[:, sl], pso[HD:HD + 1, sl])
                nc.vector.tensor_copy(po[:, sl], pso[0:HD, sl])
                nc.vector.reciprocal_approx_fast(out=rec[:, sl],
                                                 in_=den[:, sl])
                nc.gpsimd.partition_broadcast(rb[:, sl], rec[:, sl])
                nc.vector.tensor_tensor(
                    attnT[pr:pr + 64, h // 2, q0 + s * w:q0 + (s + 1) * w],
                    po[:, sl], rb[:, sl], MULT)

        # ---- fill schedule: flat f = 16*(8*qh+h)+kc -> PE work emitted
        # after that iteration's scores/exp, sized so the PE never drops
        # below the ACT rate once the exp stream is running ----
        fills = {}

        def add(f, t):
            fills.setdefault(f, []).append(t)

        for kc in range(NKC):
            add(kc + 1, lambda kc=kc: v_chunk(kc))
        add(7, lambda: kq_half(wk_sb, ktp, 0, 1, 0))
        add(11, lambda: kq_half(wk_sb, ktp, 0, 1, 1))
        # zero rows 64-127 of head slots 1-7, spread through the early
        # fills so the 1.8us memsets never clump ahead of on-path copies
        for j in range(1, HL):
            add(2 + 8 * j, lambda j=j: (
                nc.gpsimd.memset(ktp[64:P, j, :], 0.0),
                nc.vector.memset(qtp[64:P, j, :], 0.0)))
        # qh0 h1-h5: K/Q projection pairs against their deadlines
        ph1_fills = [
            (17, wk_sb, 1, 0, 0), (19, wk_sb, 1, 0, 1),
            (22, wq_sb, 1, 0, 0), (25, wq_sb, 1, 0, 1),
            (34, wk_sb, 1, 1, 0), (36, wk_sb, 1, 1, 1),
            (42, wk_sb, 2, 0, 0), (45, wk_sb, 2, 0, 1),
            (50, wq_sb, 2, 0, 0), (53, wq_sb, 2, 0, 1),
            (57, wk_sb, 2, 1, 0), (60, wk_sb, 2, 1, 1),
            (66, wk_sb, 3, 0, 0), (69, wk_sb, 3, 0, 1),
            (82, wq_sb, 3, 0, 0), (85, wq_sb, 3, 0, 1),
        ]
        for f, w_sb, oc, sh, q2 in ph1_fills:
            dst = ktp if w_sb is wk_sb else qtp
            add(f, lambda w_sb=w_sb, dst=dst, oc=oc, sh=sh, q2=q2:
                kq_half(w_sb, dst, oc, sh, q2))
        # phase 2 (>= f96): Q sh1 projections + qh0 out-proj on the ring
        # phase-2 fill iterations avoid f % 3 == 0 (no exp fires there, and
        # fills before a tile's third unit would delay its exp)
        add(98, lambda: kq_half(wk_sb, ktp, 3, 1, 0, ring=True))
        add(102, lambda: kq_half(wk_sb, ktp, 3, 1, 1, ring=True))
        ph2_kq = [(110, 0, 0), (120, 0, 1), (125, 1, 0), (132, 1, 1),
                  (162, 2, 0), (168, 2, 1), (177, 3, 0), (183, 3, 1)]
        for f, oc, q2 in ph2_kq:
            add(f, lambda oc=oc, q2=q2:
                kq_half(wq_sb, qtp, oc, 1, q2, ring=True))
        ph2_op = [(141, 0), (150, 1), (156, 2), (192, 3),
                  (198, 4), (210, 5), (227, 6), (243, 7)]
        for f, q8 in ph2_op:
            add(f, lambda q8=q8: outproj_chunk(q8))

        # ---- emission ----
        unit_ap = {}
        pending = []

        def flush_one():
            qh, h, kc, pso = pending.pop(0)
            f = 16 * (8 * qh + h) + kc
            for qc in range(2):
                nc.tensor.matmul(
                    pso[:, qc * 512:(qc + 1) * 512],
                    lhsT=vp[:, kc, h, :],
                    rhs=unit_ap.pop(2 * f + qc),
                    start=(kc == 0), stop=(kc == NKC - 1))
            if kc == NKC - 1:
                normalize(qh, h, pso,
                          split=(4 if (qh, h) == (1, HL - 1) else 1))

        # only K/Q pair-0 sh0 precedes the attention loop, in 256-col
        # sub-tiles so the PE starts on the first landed z block
        kq_half(wk_sb, ktp, 0, 0, 0, sub=4)
        kq_half(wk_sb, ktp, 0, 0, 1, sub=4)
        kq_half(wq_sb, qtp, 0, 0, 0, sub=4)
        kq_half(wq_sb, qtp, 0, 0, 1, sub=4)

        cur3 = None
        for qh in range(2):
            q0 = qh * NQH
            for h in range(HL):
                if (qh, h) == (0, 6):
                    # projections done: swap to the 2x[128,1536] ring and
                    # 1536-wide exps (ACT/iter drops below the PE rate)
                    p1p.release()
                    ssp.release()
                    pools["scores"] = tc.alloc_tile_pool(
                        name="pss2", bufs=2, space="PSUM")
                pso = pvp.tile([HD + 1, NQH], F32, name="pvo")
                for kc in range(NKC):
                    f = 16 * (8 * qh + h) + kc
                    due = fills.pop(f, [])
                    if f < PH2_F:
                        ps = pools["scores"].tile([P, NQH], F32, name="pss",
                                                  tag="pss")
                        for qc in range(2):
                            nc.tensor.matmul(
                                ps[:, qc * 512:(qc + 1) * 512],
                                lhsT=ktp[:, h, kc * P:(kc + 1) * P],
                                rhs=qtp[:, h,
                                        q0 + qc * 512:q0 + (qc + 1) * 512])
                        es_t = esp.tile([P, 1536], BF16, name="es_t",
                                        tag="es")
                        nc.scalar.activation(es_t[:, 0:1024], ps[:], EXP,
                                             scale=SCALE)
                        unit_ap[2 * f] = es_t[:, 0:512]
                        unit_ap[2 * f + 1] = es_t[:, 512:1024]
                        for t in due:
                            t()
                    else:
                        for qc in range(2):
                            pu = 2 * f + qc - 2 * PH2_F
                            # opening pair keeps the final tile full (320
                            # units = 2 + 106*3), so the last exp fires
                            # in-loop and the tail is never exp-gated
                            pos = pu if pu < 2 else (pu - 2) % 3
                            wid = 2 if pu < 2 else 3
                            if pos == 0:
                                cur3 = pools["scores"].tile(
                                    [P, 1536], F32, name="pss", tag="pss")
                            nc.tensor.matmul(
                                cur3[:, pos * 512:(pos + 1) * 512],
                                lhsT=ktp[:, h, kc * P:(kc + 1) * P],
                                rhs=qtp[:, h,
                                        q0 + qc * 512:q0 + (qc + 1) * 512])
                            if pos == wid - 1:
                                es_t = esp.tile([P, 1536], BF16, name="es_t",
                                                tag="es")
                                nc.scalar.activation(
                                    es_t[:, 0:wid * 512],
                                    cur3[:, 0:wid * 512], EXP, scale=SCALE)
                                u = 2 * f + qc
                                for k in range(wid):
                                    unit_ap[u - wid + 1 + k] = \
                                        es_t[:, k * 512:(k + 1) * 512]
                                for t in due:
                                    t()
                                due = []
                        for t in due:
                            t()
                    pending.append((qh, h, kc, pso))
                    skew = 2 if (qh, h) == (1, HL - 1) else SKEW
                    while len(pending) > skew:
                        flush_one()
        while pending:
            flush_one()

        # tail: qh1 out-projection chunks drain over three DMA queues,
        # alternating between the freed PV psum banks and the scores ring
        # (3 effective psum slots) so no chunk waits for the ring to drain
        # its last exps
        for q8 in range(8, 16):
            if q8 % 2 == 0:
                psf = pvp.tile([P, D], F32, name="pof8", tag="pvo")
                for oc2 in range(2):
                    for dc in range(HC):
                        nc.tensor.matmul(
                            psf[:, oc2 * 512:(oc2 + 1) * 512],
                            lhsT=attnT[:, dc, q8 * P:(q8 + 1) * P],
                            rhs=wo_sb[:, dc, oc2 * 512:(oc2 + 1) * 512],
                            start=(dc == 0), stop=(dc == HC - 1))
                ot = outp.tile([P, D], BF16)
                nc.vector.tensor_copy(ot[:], psf[:])
                eng = (nc.sync, nc.gpsimd, nc.scalar)[q8 % 3]
                eng.dma_start(out_d[q8 * P:(q8 + 1) * P, :], ot[:])
            else:
                outproj_chunk(q8, split_finish=(q8 == 15))

        for pool in (pools["scores"], wp, zp, outp, nrm, esp, pvp, pp):
            pool.release()

    nc.compile()
    return nc


_NC_CACHE = None


def _get_nc():
    global _NC_CACHE
    if _NC_CACHE is None:
        _NC_CACHE = _build()
    return _NC_CACHE


def _run(z, w_q, w_k, w_v, w_o, b_o, **spmd_kwargs):
    z = np.asarray(z, dtype=np.float32)
    w_q = np.asarray(w_q, dtype=np.float32)
    w_k = np.asarray(w_k, dtype=np.float32)
    w_v = np.asarray(w_v, dtype=np.float32)
    w_o = np.asarray(w_o, dtype=np.float32)
    b_o = np.asarray(b_o, dtype=np.float32)
    assert z.shape == (B, N, D)

    if not spmd_kwargs.get("trace"):
        # A stray BASS_TRACE in the environment would route through the NTFF
        # hook (absent in this image) and crash; force the no-trace path.
        os.environ["BASS_NEVER_TRACE"] = "1"

    nc = _get_nc()
    # DMA-native packings (host-side layout only, no FLOPs)
    zt = [np.ascontiguousarray(
        z[b].T.reshape(DC, P, NB, P).transpose(1, 2, 0, 3)).astype(BF)
        for b in range(B)]
    wq_h = [np.ascontiguousarray(
        w_q[:, g * DH:(g + 1) * DH].reshape(DC, P, OC, P)
        .transpose(1, 2, 0, 3)).astype(BF) for g in range(2)]
    wk_h = [np.ascontiguousarray(
        w_k[:, g * DH:(g + 1) * DH].reshape(DC, P, OC, P)
        .transpose(1, 2, 0, 3)).astype(BF) for g in range(2)]
    wv_h = [np.ascontiguousarray(
        w_v[:, g * DH:(g + 1) * DH].reshape(DC, P, DH)
        .transpose(1, 0, 2)).astype(BF) for g in range(2)]
    wo_h = [np.ascontiguousarray(
        w_o[g * DH:(g + 1) * DH, :].reshape(HC, P, D)
        .transpose(1, 0, 2)).astype(BF) for g in range(2)]
    in_maps = []
    for c in range(N_CORES):
        b, g = c // 2, c % 2
        in_maps.append({"zt": zt[b], "wq": wq_h[g], "wk": wk_h[g],
                        "wv": wv_h[g], "wo": wo_h[g]})

    res = run_bass_kernel_spmd(nc, in_maps, core_ids=list(range(N_CORES)),
                               **spmd_kwargs)
    out = np.empty((B, N, D), dtype=np.float32)
    for b in range(B):
        out[b] = res.results[2 * b]["out"].astype(np.float32)
        out[b] += res.results[2 * b + 1]["out"].astype(np.float32)
        out[b] += b_o[None, :]
    return out, res


def kernel(z, w_q, w_k, w_v, w_o, b_o):
    out, _ = _run(z, w_q, w_k, w_v, w_o, b_o)
    return out
